# revision 1
# baseline (speedup 1.0000x reference)
"""Cross-attention layer (vision<->text) on 8 Trainium2 NeuronCores.

Problem: B=16, Sv=St=1024, D=1024, fp32.
  q = vision @ Wq.T + bq            [B,Sv,D]
  k = text   @ Wk.T + bk            [B,St,D]
  v = text   @ Wv.T + bv            [B,St,D]
  scores = q @ k.T / sqrt(D)        [B,Sv,St]
  attn = softmax(scores, -1)
  cross_vision = attn @ v           [B,Sv,D]
  cross_text   = attn.T @ vision    [B,St,D]

Sharding: pure data-parallel over batch, 2 items per core, no collectives.

Per-core kernel design (per batch item):
  - Host pre-transposes weights: wqt = Wq.T/sqrt(D) [d,e], wkt = Wk.T, wvt = Wv.T.
    The 1/sqrt(D) is folded into wqt/bq. bv is added on the host after gather
    (attn rows sum to 1, so attn @ (v0 + bv) = attn @ v0 + bv, exact).
  - On-chip PE transposes build VT[d,s] and TT[d,t] from the natural-layout
    activations, half the seq dim at a time (SBUF economy).
  - QT[e,s] = wqt.T @ VT, KT[e,t] = wkt.T @ TT (weight tile stationary),
    Vv[t,d'] = TT.T @ wvt (TT tile stationary). All matmuls run as float32r
    (fp32 bits, PE truncates to ~fp22: full-rate at N>=512, ~2^-12 rel err).
  - S[s,t] = QT.T @ KT per 128-row s-tile; E = exp(S) straight out of PSUM on
    the ACT engine with accum_out producing row sums (scores are O(+-6), no
    max-subtraction needed for fp32 exp). rinv = 1/rowsum.
  - cross_vision s-tile: PE-transpose E row-block -> ET blocks, then
    CV = ET.T @ Vv accumulated over t-tiles, scaled by rinv at PSUM evac.
  - E is then scaled in-place by rinv (making attn rows), and
    cross_text = E.T @ V accumulated over s-tiles with raw V streamed back in.
"""

import sys

import numpy as np

if "/opt/trn_rl_repo" not in sys.path:
    sys.path.insert(0, "/opt/trn_rl_repo")

import concourse.bass as bass
import concourse.tile as tile
from concourse import bacc
from concourse import mybir

PHASE_MARKS = []  # (phase_name, first_unused_instruction_id) at each boundary

P = 128
B, SEQ, DIM = 16, 1024, 1024
N_CORES = 8
BPC = B // N_CORES  # batch items per core
NT = DIM // P  # 8 tiles of 128 along d/e
F32 = mybir.dt.float32
F32R = mybir.dt.float32r
AF = mybir.ActivationFunctionType
H = 512  # half of a seq dim / PSUM-bank-sized chunk


def _emit(tc, ident, vis, txt, wqt, wkt, wvt, bq_sb, bk_sb, cv_d, ct_d, pools, b):
    nc = tc.nc

    def mark(name):
        nid = nc._state.next_id()
        PHASE_MARKS.append((f"b{b}_{name}", nid))

    (p_act, p_kt, p_qt, p_vv, p_etb, p_wc, p_vvt, p_in, p_cvs, p_cts, p_vt,
     p_rp, p_rv, pp_t, pp_mm) = pools

    kt = p_kt.tile([P, NT, SEQ], F32R, name="kt", tag="kt")
    vv = p_vv.tile([P, NT, SEQ], F32R, name="vv", tag="vv")
    qt = p_qt.tile([P, NT, SEQ], F32R, name="qt", tag="qt")

    def prep(src_d):
        """Transpose the full [SEQ, DIM] tensor into actT[d_in, d_out, seq].

        actT shares its pool slot with e_sb (disjoint lifetimes within an
        item: actT dies after projQ, e_sb is born in phase F).
        """
        actT = p_act.tile([P, NT, SEQ], F32R, name="actT", tag="act_e")
        for l in range(NT):
            for hh in range(2):  # two [128, 512] half-row loads, dual queue
                tin = p_in.tile([P, H], F32R, name="tin", tag="xin")
                eng = nc.sync if hh == 0 else nc.scalar
                eng.dma_start(
                    out=tin,
                    in_=src_d[b, l * P:(l + 1) * P, hh * H:(hh + 1) * H].bitcast(F32R))
                tp4 = pp_t.tile([P, 4, P], F32R, name="tp4", tag="tp4")
                for j in range(4):
                    do = hh * 4 + j
                    nc.tensor.matmul(
                        tp4[:, j, :], tin[:, j * P:(j + 1) * P], ident,
                        is_transpose=True, start=(j == 0), stop=(j == 3),
                        skip_group_check=True,
                    )
                if hh == 0:
                    nc.vector.tensor_copy(actT[:, 0:4, l * P:(l + 1) * P], tp4)
                else:
                    nc.scalar.copy(actT[:, 4:8, l * P:(l + 1) * P], tp4)
        return actT

    def proj(w_d, bias_col, actT, out_sb, on_vector):
        """out_sb[e_in, eo, s] = sum_do w[do,eo].T @ actT[:, do, :] (+bias).

        One 512KB weight-column load per eo (weight read once per item),
        16 matmuls per load across the two seq halves (2 PSUM groups).
        """
        for eo in range(NT):
            wc = p_wc.tile([P, NT, P], F32R, name="wc", tag="wc")
            nc.gpsimd.dma_start(
                out=wc,
                in_=w_d[:, eo * P:(eo + 1) * P].rearrange("(do di) e -> di do e", di=P),
            )
            pss = [pp_mm.tile([P, H], F32, name=f"ps_p{i}", tag="mm") for i in range(2)]
            for do in range(NT):
                for sh in range(2):
                    nc.tensor.matmul(pss[sh], wc[:, do, :], actT[:, do, sh * H:(sh + 1) * H],
                                     start=(do == 0), stop=(do == NT - 1))
            for sh in range(2):
                dst = out_sb[:, eo, sh * H:(sh + 1) * H]
                if on_vector:
                    nc.vector.tensor_scalar_add(dst, pss[sh], scalar1=bias_col[:, eo:eo + 1])
                else:
                    nc.scalar.add(dst, pss[sh], add=bias_col[:, eo:eo + 1])

    def proj_v(actT):
        """vv[t_in, tb, d'] = (TT.T @ wvt) via VvT then PE-transpose.

        VvT[d'-block, t] is computed with the weight columns stationary (one
        512KB load per d'-block, 16 matmuls each => Wv read once per item),
        evacuated to a small staging tile, then transposed 128x128-wise into
        the Vv[t, d'] layout cross_vision needs.
        """
        for dpo in range(NT):
            wvc = p_wc.tile([P, NT, P], F32R, name="wvc", tag="wc")
            nc.gpsimd.dma_start(
                out=wvc,
                in_=wvt[:, dpo * P:(dpo + 1) * P].rearrange("(do di) e -> di do e", di=P),
            )
            pss = [pp_mm.tile([P, H], F32, name=f"ps_v{i}", tag="mm") for i in range(2)]
            for do in range(NT):
                for th in range(2):
                    nc.tensor.matmul(pss[th], wvc[:, do, :], actT[:, do, th * H:(th + 1) * H],
                                     start=(do == 0), stop=(do == NT - 1))
            vvt_tmp = p_vvt.tile([P, SEQ], F32R, name="vvt_tmp", tag="vvt")
            for th in range(2):
                nc.scalar.copy(vvt_tmp[:, th * H:(th + 1) * H], pss[th])
            for tg in range(2):
                tp4 = pp_t.tile([P, 4, P], F32R, name="tp4v", tag="tp4")
                for j in range(4):
                    tb = tg * 4 + j
                    nc.tensor.matmul(tp4[:, j, :], vvt_tmp[:, tb * P:(tb + 1) * P], ident,
                                     is_transpose=True, start=(j == 0), stop=(j == 3),
                                     skip_group_check=True)
                nc.vector.tensor_copy(vv[:, tg * 4:(tg + 1) * 4, dpo * P:(dpo + 1) * P], tp4)

    # ---- text -> TT -> KT, Vv ----
    mark("prepT")
    actT = prep(txt)
    mark("projK")
    proj(wkt, bk_sb, actT, kt, on_vector=False)
    mark("projV")
    proj_v(actT)

    # ---- vision -> VT -> QT ----
    mark("prepV")
    actV = prep(vis)
    mark("projQ")
    proj(wqt, bq_sb, actV, qt, on_vector=True)

    # ---- phase F: scores, softmax, cross_vision (per s-tile) ----
    # Software-pipelined: the scores matmuls of s-tile so+1 are emitted
    # between exp(so) (ACT) and the E-transposes that consume it, so the
    # in-order PE never waits on the ACT engine.
    mark("F")
    e_sb = p_act.tile([P, NT, SEQ], F32R, name="e_sb", tag="act_e")
    rinv = p_rv.tile([P, NT], F32, name="rinv", tag="rinv")
    rps = {}

    def scores_stile(so):
        rp = p_rp.tile([P, 2], F32, name="rp", tag="rp")
        pss = [pp_mm.tile([P, H], F32, name=f"ps_s{i}", tag="mm") for i in range(2)]
        for eo in range(NT):
            for tc_ in range(2):
                nc.tensor.matmul(pss[tc_], qt[:, eo, so * P:(so + 1) * P],
                                 kt[:, eo, tc_ * H:(tc_ + 1) * H],
                                 start=(eo == 0), stop=(eo == NT - 1))
        for tc_ in range(2):
            nc.scalar.activation(out=e_sb[:, so, tc_ * H:(tc_ + 1) * H], in_=pss[tc_],
                                 func=AF.Exp, accum_out=rp[:, tc_:tc_ + 1])
        rps[so] = rp

    scores_stile(0)
    for so in range(NT):
        if so + 1 < NT:
            scores_stile(so + 1)
        rp = rps.pop(so)
        rsum = p_rp.tile([P, 1], F32, name="rsum", tag="rsum")
        nc.vector.tensor_add(rsum, rp[:, 0:1], rp[:, 1:2])
        nc.vector.reciprocal(rinv[:, so:so + 1], rsum)

        # ET blocks for this s-tile (transpose the *unnormalized* E row-block)
        etb = p_etb.tile([P, NT, P], F32R, name="etb", tag="etb")
        for tg in range(2):
            tp4 = pp_t.tile([P, 4, P], F32R, name="tp4e", tag="tp4")
            for j in range(4):
                tt = tg * 4 + j
                nc.tensor.matmul(tp4[:, j, :], e_sb[:, so, tt * P:(tt + 1) * P], ident,
                                 is_transpose=True, start=(j == 0), stop=(j == 3),
                                 skip_group_check=True)
            nc.vector.tensor_copy(etb[:, tg * 4:(tg + 1) * 4, :], tp4)

        # normalize this E row-block in place (for cross_text later)
        nc.vector.tensor_scalar_mul(e_sb[:, so, :], e_sb[:, so, :],
                                    scalar1=rinv[:, so:so + 1])

        # cross_vision[s-tile] = rinv * (ET.T @ Vv)
        cvs = p_cvs.tile([P, DIM], F32, name="cvs", tag="cvs")
        pcv = [pp_mm.tile([P, H], F32, name=f"ps_cv{i}", tag="mm") for i in range(2)]
        for tt in range(NT):
            for dc in range(2):
                nc.tensor.matmul(pcv[dc], etb[:, tt, :], vv[:, tt, dc * H:(dc + 1) * H],
                                 start=(tt == 0), stop=(tt == NT - 1))
        for dc in range(2):
            nc.scalar.mul(cvs[:, dc * H:(dc + 1) * H], pcv[dc], mul=rinv[:, so:so + 1])
        nc.gpsimd.dma_start(out=cv_d[b, so * P:(so + 1) * P, :], in_=cvs)

    # ---- phase H: cross_text = E'.T @ V (E' already rinv-scaled) ----
    # 8 concurrent PSUM accumulation groups (6 from pmm + 2 borrowed from the
    # idle transpose pool): each V tile load feeds 8 matmuls and V is read
    # only once per d'-half. Loads alternate between the two HWDGE queues.
    mark("H")
    for dc in range(2):
        pss = [pp_mm.tile([P, H], F32, name=f"ps_ct{i}", tag="mm") for i in range(6)]
        pss += [pp_t.tile([P, H], F32, name=f"ps_ct{i + 6}", tag="tp4") for i in range(2)]
        for so in range(NT):
            vt = p_vt.tile([P, H], F32R, name="vt", tag="vt")
            eng = nc.sync if so % 2 == 0 else nc.scalar
            eng.dma_start(out=vt, in_=vis[b, so * P:(so + 1) * P, dc * H:(dc + 1) * H].bitcast(F32R))
            for tt in range(NT):
                nc.tensor.matmul(pss[tt], e_sb[:, so, tt * P:(tt + 1) * P], vt,
                                 start=(so == 0), stop=(so == NT - 1))
        for tt in range(NT):
            cts = p_cts.tile([P, H], F32, name="cts", tag="cts")
            if tt % 2 == 0:
                nc.vector.tensor_copy(cts, pss[tt])
            else:
                nc.scalar.copy(cts, pss[tt])
            nc.gpsimd.dma_start(out=ct_d[b, tt * P:(tt + 1) * P, dc * H:(dc + 1) * H],
                                  in_=cts)
    mark("end")


def build_nc():
    nc = bacc.Bacc("TRN2", target_bir_lowering=False, debug=False, num_devices=N_CORES)
    vis = nc.dram_tensor("vision", [BPC, SEQ, DIM], F32, kind="ExternalInput").ap()
    txt = nc.dram_tensor("text", [BPC, SEQ, DIM], F32, kind="ExternalInput").ap()
    wqt = nc.dram_tensor("wqt", [DIM, DIM], F32R, kind="ExternalInput").ap()
    wkt = nc.dram_tensor("wkt", [DIM, DIM], F32R, kind="ExternalInput").ap()
    wvt = nc.dram_tensor("wvt", [DIM, DIM], F32R, kind="ExternalInput").ap()
    bq_d = nc.dram_tensor("bq", [DIM], F32, kind="ExternalInput").ap()
    id_d = nc.dram_tensor("ident128", [P, P], F32R, kind="ExternalInput").ap()
    bk_d = nc.dram_tensor("bk", [DIM], F32, kind="ExternalInput").ap()
    cv_d = nc.dram_tensor("cross_vision", [BPC, SEQ, DIM], F32, kind="ExternalOutput").ap()
    ct_d = nc.dram_tensor("cross_text", [BPC, SEQ, DIM], F32, kind="ExternalOutput").ap()

    with tile.TileContext(nc) as tc:
        pools = []
        import contextlib
        with contextlib.ExitStack() as ctx:
            def sp(name, bufs):
                return ctx.enter_context(tc.tile_pool(name=name, bufs=bufs))

            p_act = sp("act", 1)
            p_kt = sp("kt", 1)
            p_qt = sp("qt", 1)
            p_vv = sp("vv", 1)
            p_etb = sp("etb", 1)
            p_wc = sp("wc", 3)
            p_vvt = sp("vvt", 2)
            p_in = sp("xin", 4)
            p_cvs = sp("cvs", 2)
            p_cts = sp("cts", 4)
            p_vt = sp("vt", 4)
            p_rp = sp("rp", 4)
            p_rv = sp("rv", 2)
            p_sm = sp("sm", 1)
            pp_t = ctx.enter_context(
                tc.tile_pool(name="pp_t", bufs=2, space=bass.MemorySpace.PSUM))
            pp_mm = ctx.enter_context(
                tc.tile_pool(name="pp_mm", bufs=6, space=bass.MemorySpace.PSUM))

            ident = p_sm.tile([P, P], F32R, name="ident")
            nc.sync.dma_start(out=ident, in_=id_d)
            bq_sb = p_sm.tile([P, NT], F32, name="bq_sb")
            nc.sync.dma_start(out=bq_sb, in_=bq_d.rearrange("(eo ei) -> ei eo", ei=P))
            bk_sb = p_sm.tile([P, NT], F32, name="bk_sb")
            nc.sync.dma_start(out=bk_sb, in_=bk_d.rearrange("(eo ei) -> ei eo", ei=P))

            pools = (p_act, p_kt, p_qt, p_vv, p_etb, p_wc, p_vvt, p_in,
                     p_cvs, p_cts, p_vt, p_rp, p_rv, pp_t, pp_mm)
            for b in range(BPC):
                _emit(tc, ident, vis, txt, wqt, wkt, wvt, bq_sb, bk_sb,
                      cv_d, ct_d, pools, b)
    nc.compile()
    return nc


_NC_CACHE = None


def _get_nc():
    global _NC_CACHE
    if _NC_CACHE is None:
        _NC_CACHE = build_nc()
    return _NC_CACHE


def make_in_maps(vision_repr, text_repr, Wq, bq, Wk, bk, Wv, bv):
    s = 1.0 / np.sqrt(np.float32(DIM))
    wqt = np.ascontiguousarray(np.asarray(Wq, np.float32).T * s)
    wkt = np.ascontiguousarray(np.asarray(Wk, np.float32).T)
    wvt = np.ascontiguousarray(np.asarray(Wv, np.float32).T)
    bq_s = np.asarray(bq, np.float32) * s
    bk_ = np.asarray(bk, np.float32)
    vis = np.asarray(vision_repr, np.float32)
    txt = np.asarray(text_repr, np.float32)
    in_maps = []
    for c in range(N_CORES):
        in_maps.append({
            "vision": vis[c * BPC:(c + 1) * BPC],
            "text": txt[c * BPC:(c + 1) * BPC],
            "wqt": wqt, "wkt": wkt, "wvt": wvt,
            "bq": bq_s, "bk": bk_,
            "ident128": np.eye(P, dtype=np.float32),
        })
    return in_maps


def kernel(vision_repr, text_repr, Wq, bq, Wk, bk, Wv, bv):
    from concourse.bass_utils import run_bass_kernel_spmd

    nc = _get_nc()
    in_maps = make_in_maps(vision_repr, text_repr, Wq, bq, Wk, bk, Wv, bv)
    res = run_bass_kernel_spmd(nc, in_maps, list(range(N_CORES))).results
    cv = np.concatenate([r_["cross_vision"] for r_ in res], axis=0)
    ct = np.concatenate([r_["cross_text"] for r_ in res], axis=0)
    cv = cv + np.asarray(bv, np.float32)[None, None, :]
    return cv, ct



# revision 2
# speedup vs baseline: 1.2163x; 1.2163x over previous
"""Cross-attention layer (vision<->text) on 8 Trainium2 NeuronCores.

Problem: B=16, Sv=St=1024, D=1024, fp32.
  q = vision @ Wq.T + bq            [B,Sv,D]
  k = text   @ Wk.T + bk            [B,St,D]
  v = text   @ Wv.T + bv            [B,St,D]
  scores = q @ k.T / sqrt(D)        [B,Sv,St]
  attn = softmax(scores, -1)
  cross_vision = attn @ v           [B,Sv,D]
  cross_text   = attn.T @ vision    [B,St,D]

Sharding: pure data-parallel over batch, 2 items per core, no collectives.

Algebraic restructure (q and k are never outputs):
  scores = vision @ A @ text.T + u 1^T + 1 w^T + c, with
    A = Wq.T @ Wk / sqrt(D)   (host-precomputed, fp64)
    u[s] = vision @ Wq.T @ bk / sqrt(D),  w[t] = text @ Wk.T @ bq / sqrt(D)
  The u[s] + c terms are constant along the softmax axis t, so they cancel
  exactly in softmax.  Only w[t] survives; host precomputes w = text @
  (Wk.T bq)/sqrt(D) and the device folds it in as the per-partition bias of
  the exp() (scores are built t-on-partitions).  bv is added on the host
  after gather (attn rows sum to 1, exact).  This removes one full GEMM per
  item (Q-proj + K-proj + scores -> A-proj + scores).

Per-core kernel design (per batch item; all matmuls float32r, full PE rate):
  prep:  PE-transpose text -> TT[d,t], vision -> VT[d,s]   (128 transposes)
  Vproj: V[t,d'] = TT.T @ wvt, TT tile stationary -> natural [t,d'] layout
         (no transposes needed, unlike a weight-stationary Vv^T approach)
  Hproj: HT[e,t] = A2-columns.T @ TT   (A2 = A.T streamed per 128-col block)
  ST:    ST[t,s] = HT.T @ VT per 128-row t-tile; E2 = exp(ST + w[t]) straight
         out of PSUM on ACT (scores are O(+-6), fp32 exp needs no max-sub)
  P5:    per s-tile: PE-transpose E2 column block -> E2T[s,t] blocks, evac'd
         with accum_out giving row sums -> rinv; E2T scaled by rinv in place
         (-> attn.T rows); CV[s,:] = E2.T @ V accumulated over t-tiles,
         scaled by rinv at PSUM evac
  CT:    cross_text = E2T_normalized.T @ vision accumulated over s-tiles,
         raw vision streamed back in, 8 concurrent PSUM groups
"""

import sys

import numpy as np

if "/opt/trn_rl_repo" not in sys.path:
    sys.path.insert(0, "/opt/trn_rl_repo")

import concourse.bass as bass
import concourse.tile as tile
from concourse import bacc
from concourse import mybir

PHASE_MARKS = []  # (phase_name, first_unused_instruction_id) at each boundary

P = 128
B, SEQ, DIM = 16, 1024, 1024
N_CORES = 8
BPC = B // N_CORES  # batch items per core
NT = DIM // P  # 8 tiles of 128 along d/e
F32 = mybir.dt.float32
F32R = mybir.dt.float32r
AF = mybir.ActivationFunctionType
H = 512  # half of a seq dim / PSUM-bank-sized chunk


def _emit(tc, ident, vis_d, txt_d, a2_d, wvt_sb, wcol_d, cv_d, ct_d, pools, b):
    nc = tc.nc

    def mark(name):
        nid = nc._state.next_id()
        PHASE_MARKS.append((f"b{b}_{name}", nid))

    (p_tt, p_vt, p_ht, p_v, p_wc, p_in, p_cvs, p_cts, p_vts,
     p_rp, p_rv, pp_t, pp_mm) = pools

    # per-item softmax bias column (w[t] rearranged t -> [ti, tt])
    wcol_sb = p_rv.tile([P, NT], F32, name="wcol_sb", tag="wcol")
    nc.sync.dma_start(out=wcol_sb,
                      in_=wcol_d[b].rearrange("(tt ti) -> ti tt", ti=P))

    def prep(src_d, dst, q0):
        """Transpose the full [SEQ, DIM] tensor into dst[d_in, d_out, seq]."""
        for l in range(NT):
            tin = p_in.tile([P, DIM], F32R, name="tin", tag="xin")
            eng = nc.sync if (l + q0) % 2 == 0 else nc.scalar
            eng.dma_start(out=tin, in_=src_d[b, l * P:(l + 1) * P, :].bitcast(F32R))
            for tg in range(2):
                tp4 = pp_t.tile([P, 4, P], F32R, name="tp4", tag="tp4")
                for j in range(4):
                    c = tg * 4 + j
                    nc.tensor.matmul(
                        tp4[:, j, :], tin[:, c * P:(c + 1) * P], ident,
                        is_transpose=True, start=(j == 0), stop=(j == 3),
                        skip_group_check=True,
                    )
                if tg == 0:
                    nc.vector.tensor_copy(dst[:, 0:4, l * P:(l + 1) * P], tp4)
                else:
                    nc.scalar.copy(dst[:, 4:8, l * P:(l + 1) * P], tp4)

    # ---- prep both activations (DMAs race ahead on dual queues) ----
    mark("prep")
    tt_sb = p_tt.tile([P, NT, SEQ], F32R, name="tt_sb", tag="tt_e2")
    vt_sb = p_vt.tile([P, NT, SEQ], F32R, name="vt_sb", tag="vt")
    prep(txt_d, tt_sb, 0)
    prep(vis_d, vt_sb, 1)

    # ---- V[t,d'] = TT.T @ wvt (TT tile stationary, wvt moving from SBUF) ----
    mark("projV")
    v_sb = p_v.tile([P, NT, SEQ], F32R, name="v_sb", tag="v")
    for tt in range(NT):
        psv = [pp_mm.tile([P, H], F32, name=f"ps_v{i}", tag="mm") for i in range(2)]
        for do in range(NT):
            for dc in range(2):
                nc.tensor.matmul(psv[dc], tt_sb[:, do, tt * P:(tt + 1) * P],
                                 wvt_sb[:, do, dc * H:(dc + 1) * H],
                                 start=(do == 0), stop=(do == NT - 1))
        for dc in range(2):
            if dc == 0:
                nc.vector.tensor_copy(v_sb[:, tt, dc * H:(dc + 1) * H], psv[dc])
            else:
                nc.scalar.copy(v_sb[:, tt, dc * H:(dc + 1) * H], psv[dc])

    # ---- HT[e,t] = A2-cols.T @ TT (A2 column block streamed per eo) ----
    mark("projH")
    ht_sb = p_ht.tile([P, NT, SEQ], F32R, name="ht_sb", tag="ht_e2t")
    for eo in range(NT):
        wc = p_wc.tile([P, NT, P], F32R, name="wc", tag="wc")
        nc.gpsimd.dma_start(
            out=wc,
            in_=a2_d[:, eo * P:(eo + 1) * P].rearrange("(do di) e -> di do e", di=P),
        )
        psh = [pp_mm.tile([P, H], F32, name=f"ps_h{i}", tag="mm") for i in range(2)]
        for do in range(NT):
            for th in range(2):
                nc.tensor.matmul(psh[th], wc[:, do, :],
                                 tt_sb[:, do, th * H:(th + 1) * H],
                                 start=(do == 0), stop=(do == NT - 1))
        for th in range(2):
            if th == 0:
                nc.vector.tensor_copy(ht_sb[:, eo, th * H:(th + 1) * H], psh[th])
            else:
                nc.scalar.copy(ht_sb[:, eo, th * H:(th + 1) * H], psh[th])

    # ---- ST[t,s] = HT.T @ VT; E2 = exp(ST + w[t]) (t on partitions) ----
    mark("ST")
    e2_sb = p_tt.tile([P, NT, SEQ], F32R, name="e2_sb", tag="tt_e2")
    for tt in range(NT):
        pst = [pp_mm.tile([P, H], F32, name=f"ps_s{i}", tag="mm") for i in range(2)]
        for eo in range(NT):
            for sh in range(2):
                nc.tensor.matmul(pst[sh], ht_sb[:, eo, tt * P:(tt + 1) * P],
                                 vt_sb[:, eo, sh * H:(sh + 1) * H],
                                 start=(eo == 0), stop=(eo == NT - 1))
        for sh in range(2):
            nc.scalar.activation(out=e2_sb[:, tt, sh * H:(sh + 1) * H], in_=pst[sh],
                                 func=AF.Exp, bias=wcol_sb[:, tt:tt + 1])

    # ---- P5: per s-tile: E2T blocks + row sums -> rinv; normalize; CV ----
    mark("P5")
    e2t_sb = p_ht.tile([P, NT, SEQ], F32R, name="e2t_sb", tag="ht_e2t")
    rinv = p_rv.tile([P, NT], F32, name="rinv", tag="rinv")
    for so in range(NT):
        rp = p_rp.tile([P, 2], F32, name="rp", tag="rp")
        for tg in range(2):
            tp4 = pp_t.tile([P, 4, P], F32R, name="tp4e", tag="tp4")
            for j in range(4):
                tt = tg * 4 + j
                nc.tensor.matmul(tp4[:, j, :], e2_sb[:, tt, so * P:(so + 1) * P],
                                 ident, is_transpose=True, start=(j == 0),
                                 stop=(j == 3), skip_group_check=True)
            nc.scalar.activation(out=e2t_sb[:, so, tg * H:(tg + 1) * H],
                                 in_=tp4, func=AF.Identity,
                                 accum_out=rp[:, tg:tg + 1])
        rsum = p_rp.tile([P, 1], F32, name="rsum", tag="rsum")
        nc.vector.tensor_add(rsum, rp[:, 0:1], rp[:, 1:2])
        nc.vector.reciprocal(rinv[:, so:so + 1], rsum)

        # normalize this E2T row-block in place -> attn.T rows (for CT)
        nc.vector.tensor_scalar_mul(e2t_sb[:, so, :], e2t_sb[:, so, :],
                                    scalar1=rinv[:, so:so + 1])

        # cross_vision[s-tile] = rinv * (E2.T @ V)
        cvs = p_cvs.tile([P, DIM], F32, name="cvs", tag="cvs")
        pcv = [pp_mm.tile([P, H], F32, name=f"ps_cv{i}", tag="mm") for i in range(2)]
        for tt in range(NT):
            for dc in range(2):
                nc.tensor.matmul(pcv[dc], e2_sb[:, tt, so * P:(so + 1) * P],
                                 v_sb[:, tt, dc * H:(dc + 1) * H],
                                 start=(tt == 0), stop=(tt == NT - 1))
        for dc in range(2):
            nc.scalar.mul(cvs[:, dc * H:(dc + 1) * H], pcv[dc],
                          mul=rinv[:, so:so + 1])
        nc.gpsimd.dma_start(out=cv_d[b, so * P:(so + 1) * P, :], in_=cvs)

    # ---- CT: cross_text = E2T_norm.T @ vision (vision streamed back in) ----
    mark("CT")
    for dc in range(2):
        pss = [pp_mm.tile([P, H], F32, name=f"ps_ct{i}", tag="mm") for i in range(6)]
        pss += [pp_t.tile([P, H], F32, name=f"ps_ct{i + 6}", tag="tp4") for i in range(2)]
        for so in range(NT):
            vtl = p_vts.tile([P, H], F32R, name="vtl", tag="vtl")
            eng = nc.sync if so % 2 == 0 else nc.scalar
            eng.dma_start(out=vtl,
                          in_=vis_d[b, so * P:(so + 1) * P, dc * H:(dc + 1) * H].bitcast(F32R))
            for tt in range(NT):
                nc.tensor.matmul(pss[tt], e2t_sb[:, so, tt * P:(tt + 1) * P], vtl,
                                 start=(so == 0), stop=(so == NT - 1))
        for tt in range(NT):
            cts = p_cts.tile([P, H], F32, name="cts", tag="cts")
            if tt % 2 == 0:
                nc.vector.tensor_copy(cts, pss[tt])
            else:
                nc.scalar.copy(cts, pss[tt])
            nc.gpsimd.dma_start(out=ct_d[b, tt * P:(tt + 1) * P, dc * H:(dc + 1) * H],
                                in_=cts)
    mark("end")


def build_nc():
    nc = bacc.Bacc("TRN2", target_bir_lowering=False, debug=False, num_devices=N_CORES)
    vis = nc.dram_tensor("vision", [BPC, SEQ, DIM], F32, kind="ExternalInput").ap()
    txt = nc.dram_tensor("text", [BPC, SEQ, DIM], F32, kind="ExternalInput").ap()
    a2_d = nc.dram_tensor("a2", [DIM, DIM], F32R, kind="ExternalInput").ap()
    wvt_d = nc.dram_tensor("wvt", [DIM, DIM], F32R, kind="ExternalInput").ap()
    wcol_d = nc.dram_tensor("wcol", [BPC, SEQ], F32, kind="ExternalInput").ap()
    id_d = nc.dram_tensor("ident128", [P, P], F32R, kind="ExternalInput").ap()
    cv_d = nc.dram_tensor("cross_vision", [BPC, SEQ, DIM], F32, kind="ExternalOutput").ap()
    ct_d = nc.dram_tensor("cross_text", [BPC, SEQ, DIM], F32, kind="ExternalOutput").ap()

    with tile.TileContext(nc) as tc:
        import contextlib
        with contextlib.ExitStack() as ctx:
            def sp(name, bufs):
                return ctx.enter_context(tc.tile_pool(name=name, bufs=bufs))

            p_tt = sp("tt", 1)    # TT then E2 (disjoint lifetimes)
            p_vt = sp("vt", 1)    # VT
            p_ht = sp("ht", 1)    # HT then E2T (disjoint lifetimes)
            p_v = sp("v", 1)      # V
            p_wvt = sp("wvt", 1)  # persistent Wv.T
            p_wc = sp("wc", 2)    # A2 column blocks
            p_in = sp("xin", 4)   # prep [128,1024] staging
            p_cvs = sp("cvs", 2)
            p_cts = sp("cts", 3)
            p_vts = sp("vtl", 4)  # CT vision tiles
            p_rp = sp("rp", 4)
            p_rv = sp("rv", 2)
            p_sm = sp("sm", 1)
            pp_t = ctx.enter_context(
                tc.tile_pool(name="pp_t", bufs=2, space=bass.MemorySpace.PSUM))
            pp_mm = ctx.enter_context(
                tc.tile_pool(name="pp_mm", bufs=6, space=bass.MemorySpace.PSUM))

            ident = p_sm.tile([P, P], F32R, name="ident")
            nc.sync.dma_start(out=ident, in_=id_d)
            wvt_sb = p_wvt.tile([P, NT, DIM], F32R, name="wvt_sb")
            nc.gpsimd.dma_start(
                out=wvt_sb,
                in_=wvt_d.rearrange("(do di) e -> di do e", di=P))

            pools = (p_tt, p_vt, p_ht, p_v, p_wc, p_in, p_cvs, p_cts, p_vts,
                     p_rp, p_rv, pp_t, pp_mm)
            for b in range(BPC):
                _emit(tc, ident, vis, txt, a2_d, wvt_sb, wcol_d,
                      cv_d, ct_d, pools, b)
    nc.compile()
    return nc


_NC_CACHE = None


def _get_nc():
    global _NC_CACHE
    if _NC_CACHE is None:
        _NC_CACHE = build_nc()
    return _NC_CACHE


def make_in_maps(vision_repr, text_repr, Wq, bq, Wk, bk, Wv, bv):
    s = 1.0 / np.sqrt(np.float64(DIM))
    wq64 = np.asarray(Wq, np.float64)
    wk64 = np.asarray(Wk, np.float64)
    # scores = vision @ A @ text.T with A = Wq.T @ Wk / sqrt(D);
    # device wants A2 = A.T = Wk.T @ Wq / sqrt(D) (contraction-major layout)
    a2 = np.ascontiguousarray((wk64.T @ wq64 * s).astype(np.float32))
    wvt = np.ascontiguousarray(np.asarray(Wv, np.float32).T)
    # surviving softmax bias term: w[t] = text @ (Wk.T @ bq) / sqrt(D)
    g = (wk64.T @ np.asarray(bq, np.float64)) * s
    txt = np.asarray(text_repr, np.float32)
    vis = np.asarray(vision_repr, np.float32)
    wcol = (txt.astype(np.float64) @ g).astype(np.float32)  # [B, T]
    in_maps = []
    for c in range(N_CORES):
        in_maps.append({
            "vision": vis[c * BPC:(c + 1) * BPC],
            "text": txt[c * BPC:(c + 1) * BPC],
            "a2": a2, "wvt": wvt,
            "wcol": wcol[c * BPC:(c + 1) * BPC],
            "ident128": np.eye(P, dtype=np.float32),
        })
    return in_maps


def kernel(vision_repr, text_repr, Wq, bq, Wk, bk, Wv, bv):
    from concourse.bass_utils import run_bass_kernel_spmd

    nc = _get_nc()
    in_maps = make_in_maps(vision_repr, text_repr, Wq, bq, Wk, bk, Wv, bv)
    res = run_bass_kernel_spmd(nc, in_maps, list(range(N_CORES))).results
    cv = np.concatenate([r_["cross_vision"] for r_ in res], axis=0)
    ct = np.concatenate([r_["cross_text"] for r_ in res], axis=0)
    cv = cv + np.asarray(bv, np.float32)[None, None, :]
    return cv, ct
